# revision 18
# baseline (speedup 1.0000x reference)
"""Class-attention with GFSA reaction term — Trainium2 Bass kernel, 8 NeuronCores.

Math (reference):
    q,k,v = x@W{q,k,v}.T split into H=12 heads of 64
    A  = softmax(q k^T / 8)                  per (b,h), [N,N], N=577
    attn = A + lamb_h * (3*A@A - 2*A)
    out  = (attn @ v) @ Wp.T + bp

Key algebraic restructure (avoids the N^3 A@A entirely):
    out_head = (1-2l)*A@V + 3l*A@(A@V)
    With E = exp(logits) unnormalized and D = diag(rowsum(E)):
        U = E@V,  s = rowsum(E)  (one fused matmul with a ones-column on V)
        W = (1-2l)*V + 3l*diag(1/s)*U
        out_head = diag(1/s) * (E @ W)
    No softmax max-subtraction needed: logits ~ N(0,1) by construction.

Sharding: pure data-parallel over batch, 2 batches per core, no collectives.
All layout transposes are done host-side in numpy (x^T, W^T fed directly).
"""

import os
import sys

import numpy as np

for _p in ("/opt/trn_rl_repo", "/root/.axon_site", "/root/.axon_site/_ro/trn_rl_repo"):
    if _p not in sys.path and os.path.isdir(_p):
        sys.path.append(_p)

B, N, C, H, HD = 16, 577, 768, 12, 64
NCORES = 8
BPC = B // NCORES  # batches per core
SCALE = HD**-0.5
PAIRS = H // 2
CT = C // 128  # 6 c-tiles

# token tiles (start, rows)
TOK = [(t * 128, min(128, N - t * 128)) for t in range((N + 127) // 128)]
NP = N + 1  # 578: fp32r matmuls need an even moving free dim, so pad tokens
ETC = [(0, 512), (512, NP - 512)]  # ET matmul free chunks (bank-aligned, even)
XQC = [(0, 512), (512, NP - 512)]  # q/k projection free chunks (even)
PC = [(0, 384), (384, 384)]  # v / proj free chunks

MODE = os.environ.get("KERNEL_MODE", "bf16")
# zero-fill padding columns so CoreSim's uninit-read checker passes
# (HW doesn't need it: garbage lanes never reach the output)
SIM_INIT = os.environ.get("KERNEL_SIM_INIT", "0") == "1"
# which psum->sbuf copies go to the Pool engine (bisect knob)
POOL_COPIES = set(os.environ.get("KERNEL_POOL_COPIES", "").split(","))

_CACHE = {}


def _dtypes(mode):
    import concourse.mybir as mybir

    f32, f32r, bf16 = mybir.dt.float32, mybir.dt.float32r, mybir.dt.bfloat16
    if mode == "bf16":
        return dict(W=bf16, X=bf16, QK=bf16, ATT=bf16, TP=bf16, PROJ=bf16)
    if mode == "mixed":
        return dict(W=f32r, X=f32r, QK=f32r, ATT=bf16, TP=bf16, PROJ=bf16)
    if mode == "fp32r":
        return dict(W=f32r, X=f32r, QK=f32r, ATT=f32r, TP=f32, PROJ=f32r)
    if mode == "fp32":
        return dict(W=f32, X=f32, QK=f32, ATT=f32, TP=f32, PROJ=f32)
    raise ValueError(mode)


def build(mode=MODE, loop_n=1, ablate=(), probe=()):
    """Build + compile the per-core Bass program (SPMD, identical on all cores)."""
    import concourse.mybir as mybir
    import concourse.tile as tile
    from concourse import bacc
    from concourse.masks import make_identity

    D = _dtypes(mode)
    f32 = mybir.dt.float32
    Exp = mybir.ActivationFunctionType.Exp
    mult, add = mybir.AluOpType.mult, mybir.AluOpType.add

    def cast(ap):
        return ap

    nc = bacc.Bacc("TRN2", target_bir_lowering=False, debug=False, num_devices=NCORES)

    xT = nc.dram_tensor("xT", [BPC, C, N], D["X"], kind="ExternalInput")
    wqT = nc.dram_tensor("wqT", [C, C], D["W"], kind="ExternalInput")
    wkT = nc.dram_tensor("wkT", [C, C], D["W"], kind="ExternalInput")
    wvT = nc.dram_tensor("wvT", [C, C], D["W"], kind="ExternalInput")
    wpT = nc.dram_tensor("wpT", [C, C], D["PROJ"], kind="ExternalInput")
    bpr = nc.dram_tensor("bpr", [1, C], D["PROJ"], kind="ExternalInput")
    cst = nc.dram_tensor("cst", [128, 2, H], f32, kind="ExternalInput")
    onesH = nc.dram_tensor("onesH", [128, H], D["ATT"], kind="ExternalInput")
    ones1 = nc.dram_tensor("ones1", [1, 128], D["PROJ"], kind="ExternalInput")
    out = nc.dram_tensor("out", [BPC, N, C], f32, kind="ExternalOutput")

    with tile.TileContext(nc) as tc:
        # per-mode buffer depths (SBUF budget: fp32 tiles are 2x bigger)
        big = mode == "bf16"
        xb = 2 if big else 1
        qb = 2 if big else 1
        vb = 2 if mode in ("bf16", "mixed") else 1
        eb = 2 if mode in ("bf16", "mixed") else 1
        zb = 2 if mode in ("bf16", "mixed") else 1
        with (
            tc.tile_pool(name="wp", bufs=1) as wpool,
            tc.tile_pool(name="cp", bufs=1) as cpool,
            tc.tile_pool(name="xp", bufs=xb) as xpool,
            tc.tile_pool(name="qkp", bufs=qb) as qkpool,
            tc.tile_pool(name="vap", bufs=vb) as vapool,
            tc.tile_pool(name="etp", bufs=eb + 1) as etpool,
            tc.tile_pool(name="hsp", bufs=3) as hspool,
            tc.tile_pool(name="zcp", bufs=zb) as zcpool,
            tc.tile_pool(name="obp", bufs=3) as obpool,
            tc.tile_pool(name="ps_et", bufs=2, space="PSUM") as ps_et,
            tc.tile_pool(name="ps_u", bufs=1, space="PSUM") as ps_u,
            tc.tile_pool(name="ps_z", bufs=1, space="PSUM") as ps_z,
            tc.tile_pool(name="ps_io", bufs=2, space="PSUM") as ps_io,
        ):
            # ---- persistent constants / weights ----
            wq = [wpool.tile([128, C], D["W"], tag=f"wq{ct}", name=f"wq{ct}") for ct in range(CT)]
            wk = [wpool.tile([128, C], D["W"], tag=f"wk{ct}", name=f"wk{ct}") for ct in range(CT)]
            wv = [wpool.tile([128, C], D["W"], tag=f"wv{ct}", name=f"wv{ct}") for ct in range(CT)]
            wp = [wpool.tile([128, C], D["PROJ"], tag=f"wp{ct}", name=f"wp{ct}") for ct in range(CT)]
            # weights on the scalar HWDGE queue so they stream in parallel
            # with the x^T loads issued on the sync queue inside body()
            for ct in range(CT):
                sl = slice(ct * 128, (ct + 1) * 128)
                nc.scalar.dma_start(wq[ct][:], wqT[sl, :])
                nc.scalar.dma_start(wk[ct][:], wkT[sl, :])
            for ct in range(CT):
                sl = slice(ct * 128, (ct + 1) * 128)
                nc.scalar.dma_start(wv[ct][:], wvT[sl, :])
                nc.scalar.dma_start(wp[ct][:], wpT[sl, :])
            cst_sb = cpool.tile([128, 2, H], f32, tag="cst", name="cst_sb")
            nc.sync.dma_start(cst_sb[:], cst[:, :, :])
            bp_sb = cpool.tile([1, C], D["PROJ"], tag="bp", name="bp_sb")
            nc.sync.dma_start(bp_sb[:], bpr[:, :])
            ones_row = cpool.tile([1, 128], D["PROJ"], tag="ones1", name="ones_row")
            nc.sync.dma_start(ones_row[:], ones1[:, :])
            ident = cpool.tile([128, 128], D["TP"], tag="id", name="ident")
            make_identity(nc, ident[:])

            def body():
                ST = {}

                def load_x(b):
                    xt = []
                    for ct in range(CT):
                        t_ = xpool.tile([128, NP], D["X"], tag=f"xt{ct}", name=f"xt{ct}")
                        nc.sync.dma_start(t_[:, :N], xT[b, ct * 128 : (ct + 1) * 128, :])
                        if SIM_INIT:
                            nc.gpsimd.memset(t_[:, N:], 0.0)
                        xt.append(t_)
                    ST[b] = {
                        "xt": xt, "qt": [], "kt": [], "va": [], "ets": {},
                        "zc": [
                            zcpool.tile([128, N], D["PROJ"], tag=f"zc{ct}", name=f"zc{ct}")
                            for ct in range(CT)
                        ],
                    }

                def qk_one(b, qk, dtt):
                    name, w, dst = ("q", wq, ST[b]["qt"]) if qk == 0 else ("k", wk, ST[b]["kt"])
                    xt = ST[b]["xt"]
                    o = qkpool.tile([128, NP], D["QK"], tag=f"{name}{dtt}", name=f"{name}t{dtt}")
                    for c0, cl in XQC:
                        ps = ps_io.tile([128, 512], f32, tag="io", name="iops")
                        for ct in range(CT):
                            nc.tensor.matmul(
                                ps[:, :cl],
                                lhsT=w[ct][:, dtt * 128 : dtt * 128 + 128],
                                rhs=xt[ct][:, c0 : c0 + cl],
                                start=(ct == 0),
                                stop=(ct == CT - 1),
                            )
                        nc.vector.tensor_copy(o[:, c0 : c0 + cl], ps[:, :cl])
                    dst.append(o)

                def v_one(b, ti):
                    t0, rows = TOK[ti]
                    xt = ST[b]["xt"]
                    t_ = vapool.tile([128, H, HD + 2], D["ATT"], tag=f"va{ti}", name=f"va{ti}")
                    if SIM_INIT:
                        nc.gpsimd.memset(t_[:rows, :, HD + 1 :], 0.0)
                    nc.sync.dma_start(t_[:rows, :, HD : HD + 1], onesH[:rows, :])
                    for half, (m0, ml) in enumerate(PC):
                        ps = ps_io.tile([128, 512], f32, tag="io", name="iops")
                        for ct in range(CT):
                            nc.tensor.matmul(
                                ps[:rows, :ml],
                                lhsT=xt[ct][:, t0 : t0 + rows],
                                rhs=wv[ct][:, m0 : m0 + ml],
                                start=(ct == 0),
                                stop=(ct == CT - 1),
                            )
                        nc.vector.tensor_copy(
                            t_[:rows, 6 * half : 6 * half + 6, :HD],
                            ps[:rows, :ml].rearrange("p (h d) -> p h d", d=HD),
                        )
                    ST[b]["va"].append(t_)

                def eps_pair(b, p):
                    qt, kt = ST[b]["qt"], ST[b]["kt"]
                    ets = []
                    for jt, (j0, jrows) in enumerate(TOK):
                        epair = []
                        for h01 in range(2):
                            lo = 64 * h01
                            eps = ps_et.tile([128, 1024], f32, tag="et", name="eps")
                            for i0, il in ETC:
                                nc.tensor.matmul(
                                    eps[:jrows, i0 : i0 + il],
                                    lhsT=kt[p][lo : lo + 64, j0 : j0 + jrows],
                                    rhs=qt[p][lo : lo + 64, i0 : i0 + il],
                                    start=True,
                                    stop=True,
                                )
                            esb = etpool.tile(
                                [128, NP], D["ATT"], tag=f"et{jt}_{h01}", name=f"esb{jt}_{h01}"
                            )
                            nc.scalar.activation(
                                esb[:jrows, :], eps[:jrows, :NP], Exp, scale=SCALE
                            )
                            epair.append(esb)
                        ets.append(epair)
                    ST[b]["ets"][p] = ets

                def uz_pair(b, p):
                    va = ST[b]["va"]
                    ets = ST[b]["ets"].pop(p)
                    zc = ST[b]["zc"]
                    nit = len(TOK)
                    ohs = [
                        hspool.tile([128, 128], D["TP"], tag=f"oh{it}", name=f"oh{it}")
                        for it in range(nit)
                    ]
                    for h01 in range(2):
                        head = 2 * p + h01
                        c3a = cst_sb[:, 1, head : head + 1]
                        # U' = E @ [V'|1] -> U', s   (V' pre-scaled by 1-2l)
                        ups = ps_u.tile([128, nit, HD + 2], f32, tag="u", name="ups")
                        for jt in range(nit):
                            (j0, jrows) = TOK[jt]
                            for it, (t0, rows) in enumerate(TOK):
                                nc.tensor.matmul(
                                    ups[:rows, it, : HD + 2],
                                    lhsT=ets[jt][h01][:jrows, t0 : t0 + rows],
                                    rhs=va[jt][:jrows, head, : HD + 2],
                                    start=(jt == 0 and it == 0),
                                    stop=(jt == nit - 1),
                                    skip_group_check=True,
                                )
                        # W = V' + (3l/(1-2l)) * U'/s
                        ws, srs = [], []
                        for it, (t0, rows) in enumerate(TOK):
                            sr = hspool.tile([128, 1], f32, tag=f"sr{h01}_{it}", name=f"sr{h01}_{it}")
                            nc.vector.reciprocal(sr[:rows], ups[:rows, it, HD : HD + 1])
                            sr3 = hspool.tile([128, 1], f32, tag=f"sr3_{h01}_{it}", name=f"sr3_{h01}_{it}")
                            nc.vector.tensor_scalar_mul(sr3[:rows], sr[:rows], c3a[:rows])
                            w_ = hspool.tile([128, HD], D["ATT"], tag=f"wj{h01}_{it}", name=f"wj{h01}_{it}")
                            nc.vector.scalar_tensor_tensor(
                                w_[:rows],
                                ups[:rows, it, :HD],
                                sr3[:rows],
                                va[it][:rows, head, :HD],
                                op0=mult,
                                op1=add,
                            )
                            ws.append(w_)
                            srs.append(sr)
                        # Z = E @ W ; out_head = Z / s
                        zps = ps_z.tile([128, nit, HD], f32, tag="z", name="zps")
                        for jt in range(nit):
                            (j0, jrows) = TOK[jt]
                            for it, (t0, rows) in enumerate(TOK):
                                nc.tensor.matmul(
                                    zps[:rows, it, :HD],
                                    lhsT=ets[jt][h01][:jrows, t0 : t0 + rows],
                                    rhs=ws[jt][:jrows, :HD],
                                    start=(jt == 0 and it == 0),
                                    stop=(jt == nit - 1),
                                    skip_group_check=True,
                                )
                        for it, (t0, rows) in enumerate(TOK):
                            nc.vector.tensor_scalar_mul(
                                ohs[it][:rows, 64 * h01 : 64 * h01 + HD],
                                zps[:rows, it, :],
                                srs[it][:rows],
                            )
                    for it, (t0, rows) in enumerate(TOK):
                        tps = ps_io.tile([128, 128], D["TP"], tag="io", name="tps")
                        nc.tensor.transpose(
                            tps[:, :rows], ohs[it][:rows, :], ident[:rows, :rows]
                        )
                        nc.vector.tensor_copy(zc[p][:, t0 : t0 + rows], tps[:, :rows])

                def proj_one(b, it):
                    t0, rows = TOK[it]
                    zc = ST[b]["zc"]
                    ob = obpool.tile([128, C], f32, tag="ob", name="ob")
                    for half, (m0, ml) in enumerate(PC):
                        pps = ps_io.tile([128, 512], f32, tag="io", name="iops")
                        for ct in range(CT):
                            nc.tensor.matmul(
                                pps[:rows, :ml],
                                lhsT=zc[ct][:, t0 : t0 + rows],
                                rhs=wp[ct][:, m0 : m0 + ml],
                                start=(ct == 0),
                                stop=(ct == CT - 1),
                            )
                        nc.vector.tensor_copy(ob[:rows, m0 : m0 + ml], pps[:rows, :ml])
                    nc.sync.dma_start(out[b, t0 : t0 + rows, :], ob[:rows, :])

                # ---- software-pipelined schedule over the 2 batches ----
                # b0 qkv up front; b1 qkv and b0 proj are interleaved into the
                # exp-throttled pair phases so PE always has filler work and
                # ACT starts exp-ing as early as possible.
                load_x(0)
                for dtt in range(CT):
                    qk_one(0, 0, dtt)
                    qk_one(0, 1, dtt)
                for ti in range(len(TOK)):
                    v_one(0, ti)
                load_x(1)
                for p in range(PAIRS):
                    eps_pair(0, p)
                    qk_one(1, 0, p)
                    qk_one(1, 1, p)
                    if p >= 1:
                        uz_pair(0, p - 1)
                    if p >= 2:
                        v_one(1, p - 2)
                uz_pair(0, PAIRS - 1)
                v_one(1, 4)
                for p in range(PAIRS):
                    eps_pair(1, p)
                    if p >= 1:
                        uz_pair(1, p - 1)
                        proj_one(0, p - 1)
                uz_pair(1, PAIRS - 1)
                for it in range(len(TOK)):
                    proj_one(1, it)

            if loop_n > 1:
                with tc.For_i(0, loop_n, 1):
                    body()
            else:
                body()

    nc.compile()
    return nc


def _prep_in_maps(mode, x, Wq, Wk, Wv, Wp, bp, lamb):
    import ml_dtypes

    D = _dtypes(mode)
    bf16 = ml_dtypes.bfloat16

    def npdt(d):
        import concourse.mybir as mybir

        return bf16 if d == mybir.dt.bfloat16 else np.float32

    npW, npX, npPROJ = npdt(D["W"]), npdt(D["X"]), npdt(D["PROJ"])
    wqT = np.ascontiguousarray(Wq.T).astype(npW)
    wkT = np.ascontiguousarray(Wk.T).astype(npW)
    # fold (1-2l) per head into Wv: V' = (1-2l) V, then
    # W = V' + (3l/(1-2l)) U'/s with U' = E@V'.
    c1f = (1.0 - 2.0 * lamb).astype(np.float32)  # per head, |.| >= 0.3 for N(0,.1) lamb
    vscale = np.repeat(c1f, HD)  # [C], per v-column scale
    wvT = np.ascontiguousarray(Wv.T * vscale[None, :]).astype(npW)
    wpT = np.ascontiguousarray(Wp.T).astype(npPROJ)
    bpr = np.ascontiguousarray(bp.reshape(1, C)).astype(npPROJ)
    c1 = c1f
    c3 = (3.0 * lamb / c1f).astype(np.float32)
    cstv = np.ascontiguousarray(
        np.broadcast_to(np.stack([c1, c3], 0)[None], (128, 2, H))
    ).astype(np.float32)
    onesHv = np.ones((128, H), dtype=npdt(D["ATT"]))
    ones1v = np.ones((1, 128), dtype=npPROJ)
    in_maps = []
    for core in range(NCORES):
        xs = x[core * BPC : (core + 1) * BPC]
        xTv = np.ascontiguousarray(xs.transpose(0, 2, 1)).astype(npX)
        in_maps.append(
            dict(xT=xTv, wqT=wqT, wkT=wkT, wvT=wvT, wpT=wpT, bpr=bpr, cst=cstv,
                 onesH=onesHv, ones1=ones1v)
        )
    return in_maps


def kernel(x, Wq, Wk, Wv, Wp, bp, lamb):
    from concourse.bass_utils import run_bass_kernel_spmd

    x = np.asarray(x, dtype=np.float32)
    Wq = np.asarray(Wq, dtype=np.float32)
    Wk = np.asarray(Wk, dtype=np.float32)
    Wv = np.asarray(Wv, dtype=np.float32)
    Wp = np.asarray(Wp, dtype=np.float32)
    bp = np.asarray(bp, dtype=np.float32)
    lamb = np.asarray(lamb, dtype=np.float32)

    if MODE not in _CACHE:
        _CACHE[MODE] = build(MODE)
    nc = _CACHE[MODE]
    in_maps = _prep_in_maps(MODE, x, Wq, Wk, Wv, Wp, bp, lamb)
    res = run_bass_kernel_spmd(nc, in_maps, list(range(NCORES)))
    out = np.concatenate([res.results[i]["out"] for i in range(NCORES)], axis=0)
    out += bp[None, None, :]
    return out



# revision 19
# speedup vs baseline: 1.0197x; 1.0197x over previous
"""Class-attention with GFSA reaction term — Trainium2 Bass kernel, 8 NeuronCores.

Math (reference):
    q,k,v = x@W{q,k,v}.T split into H=12 heads of 64
    A  = softmax(q k^T / 8)                  per (b,h), [N,N], N=577
    attn = A + lamb_h * (3*A@A - 2*A)
    out  = (attn @ v) @ Wp.T + bp

Key algebraic restructure (avoids the N^3 A@A entirely):
    out_head = (1-2l)*A@V + 3l*A@(A@V)
    With E = exp(logits) unnormalized and D = diag(rowsum(E)):
        U = E@V,  s = rowsum(E)  (one fused matmul with a ones-column on V)
        W = (1-2l)*V + 3l*diag(1/s)*U
        out_head = diag(1/s) * (E @ W)
    No softmax max-subtraction needed: logits ~ N(0,1) by construction.

Sharding: pure data-parallel over batch, 2 batches per core, no collectives.
All layout transposes are done host-side in numpy (x^T, W^T fed directly).
"""

import os
import sys

import numpy as np

for _p in ("/opt/trn_rl_repo", "/root/.axon_site", "/root/.axon_site/_ro/trn_rl_repo"):
    if _p not in sys.path and os.path.isdir(_p):
        sys.path.append(_p)

B, N, C, H, HD = 16, 577, 768, 12, 64
NCORES = 8
BPC = B // NCORES  # batches per core
SCALE = HD**-0.5
PAIRS = H // 2
CT = C // 128  # 6 c-tiles

# token tiles (start, rows)
TOK = [(t * 128, min(128, N - t * 128)) for t in range((N + 127) // 128)]
NP = N + 1  # 578: fp32r matmuls need an even moving free dim, so pad tokens
ETC = [(0, 512), (512, NP - 512)]  # ET matmul free chunks (bank-aligned, even)
XQC = [(0, 512), (512, NP - 512)]  # q/k projection free chunks (even)
PC = [(0, 384), (384, 384)]  # v / proj free chunks

MODE = os.environ.get("KERNEL_MODE", "bf16")
# zero-fill padding columns so CoreSim's uninit-read checker passes
# (HW doesn't need it: garbage lanes never reach the output)
SIM_INIT = os.environ.get("KERNEL_SIM_INIT", "0") == "1"
# which psum->sbuf copies go to the Pool engine (bisect knob)
POOL_COPIES = set(os.environ.get("KERNEL_POOL_COPIES", "").split(","))

_CACHE = {}


def _dtypes(mode):
    import concourse.mybir as mybir

    f32, f32r, bf16 = mybir.dt.float32, mybir.dt.float32r, mybir.dt.bfloat16
    if mode == "bf16":
        return dict(W=bf16, X=bf16, QK=bf16, ATT=bf16, TP=bf16, PROJ=bf16)
    if mode == "mixed":
        return dict(W=f32r, X=f32r, QK=f32r, ATT=bf16, TP=bf16, PROJ=bf16)
    if mode == "fp32r":
        return dict(W=f32r, X=f32r, QK=f32r, ATT=f32r, TP=f32, PROJ=f32r)
    if mode == "fp32":
        return dict(W=f32, X=f32, QK=f32, ATT=f32, TP=f32, PROJ=f32)
    raise ValueError(mode)


def build(mode=MODE, loop_n=1, ablate=(), probe=()):
    """Build + compile the per-core Bass program (SPMD, identical on all cores)."""
    import concourse.mybir as mybir
    import concourse.tile as tile
    from concourse import bacc
    from concourse.masks import make_identity

    D = _dtypes(mode)
    f32 = mybir.dt.float32
    Exp = mybir.ActivationFunctionType.Exp
    mult, add = mybir.AluOpType.mult, mybir.AluOpType.add

    def cast(ap):
        return ap

    nc = bacc.Bacc("TRN2", target_bir_lowering=False, debug=False, num_devices=NCORES)

    xT = nc.dram_tensor("xT", [BPC, C, N], D["X"], kind="ExternalInput")
    wqT = nc.dram_tensor("wqT", [C, C], D["W"], kind="ExternalInput")
    wkT = nc.dram_tensor("wkT", [C, C], D["W"], kind="ExternalInput")
    wvT = nc.dram_tensor("wvT", [C, C], D["W"], kind="ExternalInput")
    wpT = nc.dram_tensor("wpT", [C, C], D["PROJ"], kind="ExternalInput")
    bpr = nc.dram_tensor("bpr", [1, C], D["PROJ"], kind="ExternalInput")
    cst = nc.dram_tensor("cst", [128, 2, H], f32, kind="ExternalInput")
    onesH = nc.dram_tensor("onesH", [128, H], D["ATT"], kind="ExternalInput")
    onesC = nc.dram_tensor("onesC", [128, H], D["ATT"], kind="ExternalInput")
    ones1 = nc.dram_tensor("ones1", [1, 128], D["PROJ"], kind="ExternalInput")
    out = nc.dram_tensor("out", [BPC, N, C], f32, kind="ExternalOutput")

    with tile.TileContext(nc) as tc:
        # per-mode buffer depths (SBUF budget: fp32 tiles are 2x bigger)
        big = mode == "bf16"
        xb = 2 if big else 1
        qb = 2 if big else 1
        vb = 2 if mode in ("bf16", "mixed") else 1
        eb = 2 if mode in ("bf16", "mixed") else 1
        zb = 2 if mode in ("bf16", "mixed") else 1
        with (
            tc.tile_pool(name="wp", bufs=1) as wpool,
            tc.tile_pool(name="cp", bufs=1) as cpool,
            tc.tile_pool(name="xp", bufs=xb) as xpool,
            tc.tile_pool(name="qkp", bufs=qb) as qkpool,
            tc.tile_pool(name="vap", bufs=vb) as vapool,
            tc.tile_pool(name="etp", bufs=eb + 1) as etpool,
            tc.tile_pool(name="hsp", bufs=3) as hspool,
            tc.tile_pool(name="zcp", bufs=zb) as zcpool,
            tc.tile_pool(name="obp", bufs=3) as obpool,
            tc.tile_pool(name="ps_et", bufs=2, space="PSUM") as ps_et,
            tc.tile_pool(name="ps_u", bufs=1, space="PSUM") as ps_u,
            tc.tile_pool(name="ps_z", bufs=1, space="PSUM") as ps_z,
            tc.tile_pool(name="ps_io", bufs=2, space="PSUM") as ps_io,
        ):
            # ---- persistent constants / weights ----
            wq = [wpool.tile([128, C], D["W"], tag=f"wq{ct}", name=f"wq{ct}") for ct in range(CT)]
            wk = [wpool.tile([128, C], D["W"], tag=f"wk{ct}", name=f"wk{ct}") for ct in range(CT)]
            wv = [wpool.tile([128, C], D["W"], tag=f"wv{ct}", name=f"wv{ct}") for ct in range(CT)]
            wp = [wpool.tile([128, C], D["PROJ"], tag=f"wp{ct}", name=f"wp{ct}") for ct in range(CT)]
            # weights on the scalar HWDGE queue so they stream in parallel
            # with the x^T loads issued on the sync queue inside body()
            for ct in range(CT):
                sl = slice(ct * 128, (ct + 1) * 128)
                nc.scalar.dma_start(wq[ct][:], wqT[sl, :])
                nc.scalar.dma_start(wk[ct][:], wkT[sl, :])
            for ct in range(CT):
                sl = slice(ct * 128, (ct + 1) * 128)
                nc.scalar.dma_start(wv[ct][:], wvT[sl, :])
                nc.scalar.dma_start(wp[ct][:], wpT[sl, :])
            cst_sb = cpool.tile([128, 2, H], f32, tag="cst", name="cst_sb")
            nc.sync.dma_start(cst_sb[:], cst[:, :, :])
            bp_sb = cpool.tile([1, C], D["PROJ"], tag="bp", name="bp_sb")
            nc.sync.dma_start(bp_sb[:], bpr[:, :])
            ones_row = cpool.tile([1, 128], D["PROJ"], tag="ones1", name="ones_row")
            nc.sync.dma_start(ones_row[:], ones1[:, :])
            ident = cpool.tile([128, 128], D["TP"], tag="id", name="ident")
            make_identity(nc, ident[:])

            def body():
                ST = {}

                def load_x(b):
                    xt = []
                    for ct in range(CT):
                        t_ = xpool.tile([128, NP], D["X"], tag=f"xt{ct}", name=f"xt{ct}")
                        nc.sync.dma_start(t_[:, :N], xT[b, ct * 128 : (ct + 1) * 128, :])
                        if SIM_INIT:
                            nc.gpsimd.memset(t_[:, N:], 0.0)
                        xt.append(t_)
                    ST[b] = {
                        "xt": xt, "qt": [], "kt": [], "va": [], "ets": {},
                        "zc": [
                            zcpool.tile([128, N], D["PROJ"], tag=f"zc{ct}", name=f"zc{ct}")
                            for ct in range(CT)
                        ],
                    }

                def qk_one(b, qk, dtt):
                    name, w, dst = ("q", wq, ST[b]["qt"]) if qk == 0 else ("k", wk, ST[b]["kt"])
                    xt = ST[b]["xt"]
                    o = qkpool.tile([128, NP], D["QK"], tag=f"{name}{dtt}", name=f"{name}t{dtt}")
                    for c0, cl in XQC:
                        ps = ps_io.tile([128, 512], f32, tag="io", name="iops")
                        for ct in range(CT):
                            nc.tensor.matmul(
                                ps[:, :cl],
                                lhsT=w[ct][:, dtt * 128 : dtt * 128 + 128],
                                rhs=xt[ct][:, c0 : c0 + cl],
                                start=(ct == 0),
                                stop=(ct == CT - 1),
                            )
                        nc.vector.tensor_copy(o[:, c0 : c0 + cl], ps[:, :cl])
                    dst.append(o)

                def v_one(b, ti):
                    t0, rows = TOK[ti]
                    xt = ST[b]["xt"]
                    t_ = vapool.tile([128, H, HD + 2], D["ATT"], tag=f"va{ti}", name=f"va{ti}")
                    nc.sync.dma_start(t_[:rows, :, HD : HD + 1], onesH[:rows, :])
                    nc.sync.dma_start(t_[:rows, :, HD + 1 : HD + 2], onesC[:rows, :])
                    for half, (m0, ml) in enumerate(PC):
                        ps = ps_io.tile([128, 512], f32, tag="io", name="iops")
                        for ct in range(CT):
                            nc.tensor.matmul(
                                ps[:rows, :ml],
                                lhsT=xt[ct][:, t0 : t0 + rows],
                                rhs=wv[ct][:, m0 : m0 + ml],
                                start=(ct == 0),
                                stop=(ct == CT - 1),
                            )
                        nc.vector.tensor_copy(
                            t_[:rows, 6 * half : 6 * half + 6, :HD],
                            ps[:rows, :ml].rearrange("p (h d) -> p h d", d=HD),
                        )
                    ST[b]["va"].append(t_)

                def eps_pair(b, p):
                    qt, kt = ST[b]["qt"], ST[b]["kt"]
                    ets = []
                    for jt, (j0, jrows) in enumerate(TOK):
                        epair = []
                        for h01 in range(2):
                            lo = 64 * h01
                            eps = ps_et.tile([128, 1024], f32, tag="et", name="eps")
                            for i0, il in ETC:
                                nc.tensor.matmul(
                                    eps[:jrows, i0 : i0 + il],
                                    lhsT=kt[p][lo : lo + 64, j0 : j0 + jrows],
                                    rhs=qt[p][lo : lo + 64, i0 : i0 + il],
                                    start=True,
                                    stop=True,
                                )
                            esb = etpool.tile(
                                [128, NP], D["ATT"], tag=f"et{jt}_{h01}", name=f"esb{jt}_{h01}"
                            )
                            nc.scalar.activation(
                                esb[:jrows, :], eps[:jrows, :NP], Exp, scale=SCALE
                            )
                            epair.append(esb)
                        ets.append(epair)
                    ST[b]["ets"][p] = ets

                def uz_pair(b, p):
                    va = ST[b]["va"]
                    ets = ST[b]["ets"].pop(p)
                    zc = ST[b]["zc"]
                    nit = len(TOK)
                    ohs = [
                        hspool.tile([128, 128], D["TP"], tag=f"oh{it}", name=f"oh{it}")
                        for it in range(nit)
                    ]
                    for h01 in range(2):
                        head = 2 * p + h01
                        # U' = E @ [V'|1] -> U', s   (V' pre-scaled by 1-2l)
                        ups = ps_u.tile([128, nit, HD + 2], f32, tag="u", name="ups")
                        for jt in range(nit):
                            (j0, jrows) = TOK[jt]
                            for it, (t0, rows) in enumerate(TOK):
                                nc.tensor.matmul(
                                    ups[:rows, it, : HD + 2],
                                    lhsT=ets[jt][h01][:jrows, t0 : t0 + rows],
                                    rhs=va[jt][:jrows, head, : HD + 2],
                                    start=(jt == 0 and it == 0),
                                    stop=(jt == nit - 1),
                                    skip_group_check=True,
                                )
                        # W = V' + (3l/(1-2l)) * U'/s
                        ws, srs = [], []
                        for it, (t0, rows) in enumerate(TOK):
                            sr = hspool.tile([128, 1], f32, tag=f"sr{h01}_{it}", name=f"sr{h01}_{it}")
                            nc.vector.reciprocal(sr[:rows], ups[:rows, it, HD : HD + 1])
                            sr3 = hspool.tile([128, 1], f32, tag=f"sr3_{h01}_{it}", name=f"sr3_{h01}_{it}")
                            nc.vector.reciprocal(sr3[:rows], ups[:rows, it, HD + 1 : HD + 2])
                            w_ = hspool.tile([128, HD], D["ATT"], tag=f"wj{h01}_{it}", name=f"wj{h01}_{it}")
                            nc.vector.scalar_tensor_tensor(
                                w_[:rows],
                                ups[:rows, it, :HD],
                                sr3[:rows],
                                va[it][:rows, head, :HD],
                                op0=mult,
                                op1=add,
                            )
                            ws.append(w_)
                            srs.append(sr)
                        # Z = E @ W ; out_head = Z / s
                        zps = ps_z.tile([128, nit, HD], f32, tag="z", name="zps")
                        for jt in range(nit):
                            (j0, jrows) = TOK[jt]
                            for it, (t0, rows) in enumerate(TOK):
                                nc.tensor.matmul(
                                    zps[:rows, it, :HD],
                                    lhsT=ets[jt][h01][:jrows, t0 : t0 + rows],
                                    rhs=ws[jt][:jrows, :HD],
                                    start=(jt == 0 and it == 0),
                                    stop=(jt == nit - 1),
                                    skip_group_check=True,
                                )
                        for it, (t0, rows) in enumerate(TOK):
                            nc.vector.tensor_scalar_mul(
                                ohs[it][:rows, 64 * h01 : 64 * h01 + HD],
                                zps[:rows, it, :],
                                srs[it][:rows],
                            )
                    for it, (t0, rows) in enumerate(TOK):
                        tps = ps_io.tile([128, 128], D["TP"], tag="io", name="tps")
                        nc.tensor.transpose(
                            tps[:, :rows], ohs[it][:rows, :], ident[:rows, :rows]
                        )
                        nc.vector.tensor_copy(zc[p][:, t0 : t0 + rows], tps[:, :rows])

                def proj_one(b, it):
                    t0, rows = TOK[it]
                    zc = ST[b]["zc"]
                    ob = obpool.tile([128, C], f32, tag="ob", name="ob")
                    for half, (m0, ml) in enumerate(PC):
                        pps = ps_io.tile([128, 512], f32, tag="io", name="iops")
                        for ct in range(CT):
                            nc.tensor.matmul(
                                pps[:rows, :ml],
                                lhsT=zc[ct][:, t0 : t0 + rows],
                                rhs=wp[ct][:, m0 : m0 + ml],
                                start=(ct == 0),
                                stop=(ct == CT - 1),
                            )
                        nc.vector.tensor_copy(ob[:rows, m0 : m0 + ml], pps[:rows, :ml])
                    nc.sync.dma_start(out[b, t0 : t0 + rows, :], ob[:rows, :])

                # ---- software-pipelined schedule over the 2 batches ----
                # b0 qkv up front; b1 qkv and b0 proj are interleaved into the
                # exp-throttled pair phases so PE always has filler work and
                # ACT starts exp-ing as early as possible.
                load_x(0)
                for dtt in range(CT):
                    qk_one(0, 0, dtt)
                    qk_one(0, 1, dtt)
                for ti in range(len(TOK)):
                    v_one(0, ti)
                load_x(1)
                for p in range(PAIRS):
                    eps_pair(0, p)
                    qk_one(1, 0, p)
                    qk_one(1, 1, p)
                    if p >= 1:
                        uz_pair(0, p - 1)
                    if p >= 2:
                        v_one(1, p - 2)
                uz_pair(0, PAIRS - 1)
                v_one(1, 4)
                for p in range(PAIRS):
                    eps_pair(1, p)
                    if p >= 1:
                        uz_pair(1, p - 1)
                        proj_one(0, p - 1)
                uz_pair(1, PAIRS - 1)
                for it in range(len(TOK)):
                    proj_one(1, it)

            if loop_n > 1:
                with tc.For_i(0, loop_n, 1):
                    body()
            else:
                body()

    nc.compile()
    return nc


def _prep_in_maps(mode, x, Wq, Wk, Wv, Wp, bp, lamb):
    import ml_dtypes

    D = _dtypes(mode)
    bf16 = ml_dtypes.bfloat16

    def npdt(d):
        import concourse.mybir as mybir

        return bf16 if d == mybir.dt.bfloat16 else np.float32

    npW, npX, npPROJ = npdt(D["W"]), npdt(D["X"]), npdt(D["PROJ"])
    wqT = np.ascontiguousarray(Wq.T).astype(npW)
    wkT = np.ascontiguousarray(Wk.T).astype(npW)
    # fold (1-2l) per head into Wv: V' = (1-2l) V, then
    # W = V' + (3l/(1-2l)) U'/s with U' = E@V'.
    c1f = (1.0 - 2.0 * lamb).astype(np.float32)  # per head, |.| >= 0.3 for N(0,.1) lamb
    vscale = np.repeat(c1f, HD)  # [C], per v-column scale
    wvT = np.ascontiguousarray(Wv.T * vscale[None, :]).astype(npW)
    wpT = np.ascontiguousarray(Wp.T).astype(npPROJ)
    bpr = np.ascontiguousarray(bp.reshape(1, C)).astype(npPROJ)
    c1 = c1f
    c3 = (3.0 * lamb / c1f).astype(np.float32)
    cstv = np.ascontiguousarray(
        np.broadcast_to(np.stack([c1, c3], 0)[None], (128, 2, H))
    ).astype(np.float32)
    onesHv = np.ones((128, H), dtype=npdt(D["ATT"]))
    onesCv = np.ascontiguousarray(
        np.broadcast_to((1.0 / c3)[None, :], (128, H))
    ).astype(npdt(D["ATT"]))
    ones1v = np.ones((1, 128), dtype=npPROJ)
    in_maps = []
    for core in range(NCORES):
        xs = x[core * BPC : (core + 1) * BPC]
        xTv = np.ascontiguousarray(xs.transpose(0, 2, 1)).astype(npX)
        in_maps.append(
            dict(xT=xTv, wqT=wqT, wkT=wkT, wvT=wvT, wpT=wpT, bpr=bpr, cst=cstv,
                 onesH=onesHv, onesC=onesCv, ones1=ones1v)
        )
    return in_maps


def kernel(x, Wq, Wk, Wv, Wp, bp, lamb):
    from concourse.bass_utils import run_bass_kernel_spmd

    x = np.asarray(x, dtype=np.float32)
    Wq = np.asarray(Wq, dtype=np.float32)
    Wk = np.asarray(Wk, dtype=np.float32)
    Wv = np.asarray(Wv, dtype=np.float32)
    Wp = np.asarray(Wp, dtype=np.float32)
    bp = np.asarray(bp, dtype=np.float32)
    lamb = np.asarray(lamb, dtype=np.float32)

    if MODE not in _CACHE:
        _CACHE[MODE] = build(MODE)
    nc = _CACHE[MODE]
    in_maps = _prep_in_maps(MODE, x, Wq, Wk, Wv, Wp, bp, lamb)
    res = run_bass_kernel_spmd(nc, in_maps, list(range(NCORES)))
    out = np.concatenate([res.results[i]["out"] for i in range(NCORES)], axis=0)
    out += bp[None, None, :]
    return out



# revision 20
# speedup vs baseline: 1.1653x; 1.1428x over previous
"""Class-attention with GFSA reaction term — Trainium2 Bass kernel, 8 NeuronCores.

Math (reference):
    q,k,v = x@W{q,k,v}.T split into H=12 heads of 64
    A  = softmax(q k^T / 8)                  per (b,h), [N,N], N=577
    attn = A + lamb_h * (3*A@A - 2*A)
    out  = (attn @ v) @ Wp.T + bp

Key algebraic restructure (avoids the N^3 A@A entirely):
    out_head = (1-2l)*A@V + 3l*A@(A@V)
    With E = exp(logits) unnormalized and D = diag(rowsum(E)):
        U = E@V,  s = rowsum(E)  (one fused matmul with a ones-column on V)
        W = (1-2l)*V + 3l*diag(1/s)*U
        out_head = diag(1/s) * (E @ W)
    No softmax max-subtraction needed: logits ~ N(0,1) by construction.

Sharding: pure data-parallel over batch, 2 batches per core, no collectives.
All layout transposes are done host-side in numpy (x^T, W^T fed directly).
"""

import os
import sys

import numpy as np

for _p in ("/opt/trn_rl_repo", "/root/.axon_site", "/root/.axon_site/_ro/trn_rl_repo"):
    if _p not in sys.path and os.path.isdir(_p):
        sys.path.append(_p)

B, N, C, H, HD = 16, 577, 768, 12, 64
NCORES = 8
BPC = B // NCORES  # batches per core
SCALE = HD**-0.5
PAIRS = H // 2
CT = C // 128  # 6 c-tiles

# token tiles (start, rows)
TOK = [(t * 128, min(128, N - t * 128)) for t in range((N + 127) // 128)]
NP = N + 1  # 578: fp32r matmuls need an even moving free dim, so pad tokens
ETC = [(0, 512), (512, NP - 512)]  # ET matmul free chunks (bank-aligned, even)
XQC = [(0, 512), (512, NP - 512)]  # q/k projection free chunks (even)
PC = [(0, 384), (384, 384)]  # v / proj free chunks

MODE = os.environ.get("KERNEL_MODE", "bf16")
# zero-fill padding columns so CoreSim's uninit-read checker passes
# (HW doesn't need it: garbage lanes never reach the output)
SIM_INIT = os.environ.get("KERNEL_SIM_INIT", "0") == "1"
# which psum->sbuf copies go to the Pool engine (bisect knob)
POOL_COPIES = set(os.environ.get("KERNEL_POOL_COPIES", "").split(","))

_CACHE = {}


def _dtypes(mode):
    import concourse.mybir as mybir

    f32, f32r, bf16 = mybir.dt.float32, mybir.dt.float32r, mybir.dt.bfloat16
    if mode == "bf16":
        return dict(W=bf16, X=bf16, QK=bf16, ATT=bf16, TP=bf16, PROJ=bf16)
    if mode == "mixed":
        return dict(W=f32r, X=f32r, QK=f32r, ATT=bf16, TP=bf16, PROJ=bf16)
    if mode == "fp32r":
        return dict(W=f32r, X=f32r, QK=f32r, ATT=f32r, TP=f32, PROJ=f32r)
    if mode == "fp32":
        return dict(W=f32, X=f32, QK=f32, ATT=f32, TP=f32, PROJ=f32)
    raise ValueError(mode)


def build(mode=MODE, loop_n=1, ablate=(), probe=()):
    """Build + compile the per-core Bass program (SPMD, identical on all cores)."""
    import concourse.mybir as mybir
    import concourse.tile as tile
    from concourse import bacc
    from concourse.masks import make_identity

    D = _dtypes(mode)
    f32 = mybir.dt.float32
    Exp = mybir.ActivationFunctionType.Exp
    mult, add = mybir.AluOpType.mult, mybir.AluOpType.add

    def cast(ap):
        return ap

    nc = bacc.Bacc("TRN2", target_bir_lowering=False, debug=False, num_devices=NCORES)

    xT = nc.dram_tensor("xT", [BPC, C, N], D["X"], kind="ExternalInput")
    wqT = nc.dram_tensor("wqT", [C, C], D["W"], kind="ExternalInput")
    wkT = nc.dram_tensor("wkT", [C, C], D["W"], kind="ExternalInput")
    wvT = nc.dram_tensor("wvT", [C, C], D["W"], kind="ExternalInput")
    wpT = nc.dram_tensor("wpT", [C, C], D["PROJ"], kind="ExternalInput")
    bpr = nc.dram_tensor("bpr", [1, C], D["PROJ"], kind="ExternalInput")
    cst = nc.dram_tensor("cst", [128, 2, H], f32, kind="ExternalInput")
    onesH = nc.dram_tensor("onesH", [128, H], D["ATT"], kind="ExternalInput")
    onesC = nc.dram_tensor("onesC", [128, H], D["ATT"], kind="ExternalInput")
    ones1 = nc.dram_tensor("ones1", [1, 128], D["PROJ"], kind="ExternalInput")
    out = nc.dram_tensor("out", [BPC, N, C], f32, kind="ExternalOutput")

    with tile.TileContext(nc) as tc:
        # per-mode buffer depths (SBUF budget: fp32 tiles are 2x bigger)
        big = mode == "bf16"
        xb = 2 if big else 1
        qb = 2 if big else 1
        vb = 2 if mode in ("bf16", "mixed") else 1
        eb = 2 if mode in ("bf16", "mixed") else 1
        zb = 2 if mode in ("bf16", "mixed") else 1
        with (
            tc.tile_pool(name="wp", bufs=1) as wpool,
            tc.tile_pool(name="cp", bufs=1) as cpool,
            tc.tile_pool(name="xp", bufs=xb) as xpool,
            tc.tile_pool(name="qkp", bufs=qb) as qkpool,
            tc.tile_pool(name="vap", bufs=vb) as vapool,
            tc.tile_pool(name="etp", bufs=eb + 1) as etpool,
            tc.tile_pool(name="hsp", bufs=3) as hspool,
            tc.tile_pool(name="zcp", bufs=zb) as zcpool,
            tc.tile_pool(name="obp", bufs=3) as obpool,
            tc.tile_pool(name="ps_et", bufs=2, space="PSUM") as ps_et,
            tc.tile_pool(name="ps_u", bufs=1, space="PSUM") as ps_u,
            tc.tile_pool(name="ps_z", bufs=1, space="PSUM") as ps_z,
            tc.tile_pool(name="ps_io", bufs=2, space="PSUM") as ps_io,
        ):
            # ---- persistent constants / weights ----
            wq = [wpool.tile([128, C], D["W"], tag=f"wq{ct}", name=f"wq{ct}") for ct in range(CT)]
            wk = [wpool.tile([128, C], D["W"], tag=f"wk{ct}", name=f"wk{ct}") for ct in range(CT)]
            wv = [wpool.tile([128, C], D["W"], tag=f"wv{ct}", name=f"wv{ct}") for ct in range(CT)]
            wp = [wpool.tile([128, C], D["PROJ"], tag=f"wp{ct}", name=f"wp{ct}") for ct in range(CT)]
            # weights on the scalar HWDGE queue so they stream in parallel
            # with the x^T loads issued on the sync queue inside body()
            for ct in range(CT):
                sl = slice(ct * 128, (ct + 1) * 128)
                nc.scalar.dma_start(wq[ct][:], wqT[sl, :])
                nc.scalar.dma_start(wk[ct][:], wkT[sl, :])
            for ct in range(CT):
                sl = slice(ct * 128, (ct + 1) * 128)
                nc.scalar.dma_start(wv[ct][:], wvT[sl, :])
                nc.scalar.dma_start(wp[ct][:], wpT[sl, :])
            cst_sb = cpool.tile([128, 2, H], f32, tag="cst", name="cst_sb")
            nc.sync.dma_start(cst_sb[:], cst[:, :, :])
            bp_sb = cpool.tile([1, C], D["PROJ"], tag="bp", name="bp_sb")
            nc.sync.dma_start(bp_sb[:], bpr[:, :])
            ones_row = cpool.tile([1, 128], D["PROJ"], tag="ones1", name="ones_row")
            nc.sync.dma_start(ones_row[:], ones1[:, :])
            ident = cpool.tile([128, 128], D["TP"], tag="id", name="ident")
            make_identity(nc, ident[:])

            def body():
                ST = {}

                def load_x(b):
                    xt = []
                    for ct in range(CT):
                        t_ = xpool.tile([128, NP], D["X"], tag=f"xt{ct}", name=f"xt{ct}")
                        nc.sync.dma_start(t_[:, :N], xT[b, ct * 128 : (ct + 1) * 128, :])
                        if SIM_INIT:
                            nc.gpsimd.memset(t_[:, N:], 0.0)
                        xt.append(t_)
                    ST[b] = {
                        "xt": xt, "qt": [], "kt": [], "va": [], "ets": {},
                        "zc": [
                            zcpool.tile([128, N], D["PROJ"], tag=f"zc{ct}", name=f"zc{ct}")
                            for ct in range(CT)
                        ],
                    }

                def qk_one(b, qk, dtt):
                    name, w, dst = ("q", wq, ST[b]["qt"]) if qk == 0 else ("k", wk, ST[b]["kt"])
                    xt = ST[b]["xt"]
                    o = qkpool.tile([128, NP], D["QK"], tag=f"{name}{dtt}", name=f"{name}t{dtt}")
                    for c0, cl in XQC:
                        ps = ps_io.tile([128, 512], f32, tag="io", name="iops")
                        for ct in range(CT):
                            nc.tensor.matmul(
                                ps[:, :cl],
                                lhsT=w[ct][:, dtt * 128 : dtt * 128 + 128],
                                rhs=xt[ct][:, c0 : c0 + cl],
                                start=(ct == 0),
                                stop=(ct == CT - 1),
                            )
                        nc.vector.tensor_copy(o[:, c0 : c0 + cl], ps[:, :cl])
                    dst.append(o)

                def v_one(b, ti):
                    t0, rows = TOK[ti]
                    xt = ST[b]["xt"]
                    t_ = vapool.tile([128, H, HD + 2], D["ATT"], tag=f"va{ti}", name=f"va{ti}")
                    nc.sync.dma_start(t_[:rows, :, HD : HD + 1], onesH[:rows, :])
                    nc.sync.dma_start(t_[:rows, :, HD + 1 : HD + 2], onesC[:rows, :])
                    for half, (m0, ml) in enumerate(PC):
                        ps = ps_io.tile([128, 512], f32, tag="io", name="iops")
                        for ct in range(CT):
                            nc.tensor.matmul(
                                ps[:rows, :ml],
                                lhsT=xt[ct][:, t0 : t0 + rows],
                                rhs=wv[ct][:, m0 : m0 + ml],
                                start=(ct == 0),
                                stop=(ct == CT - 1),
                            )
                        nc.vector.tensor_copy(
                            t_[:rows, 6 * half : 6 * half + 6, :HD],
                            ps[:rows, :ml].rearrange("p (h d) -> p h d", d=HD),
                        )
                    ST[b]["va"].append(t_)

                def eps_pair(b, p):
                    qt, kt = ST[b]["qt"], ST[b]["kt"]
                    ets = []
                    for jt, (j0, jrows) in enumerate(TOK):
                        epair = []
                        for h01 in range(2):
                            lo = 64 * h01
                            eps = ps_et.tile([128, 1024], f32, tag="et", name="eps")
                            for i0, il in ETC:
                                nc.tensor.matmul(
                                    eps[:jrows, i0 : i0 + il],
                                    lhsT=kt[p][lo : lo + 64, j0 : j0 + jrows],
                                    rhs=qt[p][lo : lo + 64, i0 : i0 + il],
                                    start=True,
                                    stop=True,
                                )
                            esb = etpool.tile(
                                [128, NP], D["ATT"], tag=f"et{jt}_{h01}", name=f"esb{jt}_{h01}"
                            )
                            nc.scalar.activation(
                                esb[:jrows, :], eps[:jrows, :NP], Exp, scale=SCALE
                            )
                            epair.append(esb)
                        ets.append(epair)
                    ST[b]["ets"][p] = ets

                def uz_pair(b, p):
                    va = ST[b]["va"]
                    ets = ST[b]["ets"].pop(p)
                    zc = ST[b]["zc"]
                    nit = len(TOK)
                    ohs = [
                        hspool.tile([128, 128], D["TP"], tag=f"oh{it}", name=f"oh{it}")
                        for it in range(nit)
                    ]
                    for h01 in range(2):
                        head = 2 * p + h01
                        # U' = E @ [V'|1] -> U', s   (V' pre-scaled by 1-2l)
                        ups = ps_u.tile([128, nit, HD + 2], f32, tag="u", name="ups")
                        for jt in range(nit):
                            (j0, jrows) = TOK[jt]
                            for it, (t0, rows) in enumerate(TOK):
                                nc.tensor.matmul(
                                    ups[:rows, it, : HD + 2],
                                    lhsT=ets[jt][h01][:jrows, t0 : t0 + rows],
                                    rhs=va[jt][:jrows, head, : HD + 2],
                                    start=(jt == 0 and it == 0),
                                    stop=(jt == nit - 1),
                                    skip_group_check=True,
                                )
                        # W = V' + (3l/(1-2l)) * U'/s.  One strided
                        # reciprocal covers all 5 tiles' s and s/c3' slots
                        # (junk rows of the short last tile are never read).
                        sra = hspool.tile([128, nit, 2], f32, tag=f"sra{h01}", name=f"sra{h01}")
                        nc.vector.reciprocal(sra[:, :, :], ups[:, :, HD : HD + 2])
                        ws, srs = [], []
                        for it, (t0, rows) in enumerate(TOK):
                            w_ = hspool.tile([128, HD], D["ATT"], tag=f"wj{h01}_{it}", name=f"wj{h01}_{it}")
                            nc.vector.scalar_tensor_tensor(
                                w_[:rows],
                                ups[:rows, it, :HD],
                                sra[:rows, it, 1:2],
                                va[it][:rows, head, :HD],
                                op0=mult,
                                op1=add,
                            )
                            ws.append(w_)
                            srs.append(sra[:, it, 0:1])
                        # Z = E @ W ; out_head = Z / s
                        zps = ps_z.tile([128, nit, HD], f32, tag="z", name="zps")
                        for jt in range(nit):
                            (j0, jrows) = TOK[jt]
                            for it, (t0, rows) in enumerate(TOK):
                                nc.tensor.matmul(
                                    zps[:rows, it, :HD],
                                    lhsT=ets[jt][h01][:jrows, t0 : t0 + rows],
                                    rhs=ws[jt][:jrows, :HD],
                                    start=(jt == 0 and it == 0),
                                    stop=(jt == nit - 1),
                                    skip_group_check=True,
                                )
                        for it, (t0, rows) in enumerate(TOK):
                            nc.vector.tensor_scalar_mul(
                                ohs[it][:rows, 64 * h01 : 64 * h01 + HD],
                                zps[:rows, it, :],
                                srs[it][:rows],
                            )
                    for it, (t0, rows) in enumerate(TOK):
                        tps = ps_io.tile([128, 128], D["TP"], tag="io", name="tps")
                        nc.tensor.transpose(
                            tps[:, :rows], ohs[it][:rows, :], ident[:rows, :rows]
                        )
                        nc.vector.tensor_copy(zc[p][:, t0 : t0 + rows], tps[:, :rows])

                def proj_one(b, it):
                    t0, rows = TOK[it]
                    zc = ST[b]["zc"]
                    ob = obpool.tile([128, C], f32, tag="ob", name="ob")
                    for half, (m0, ml) in enumerate(PC):
                        pps = ps_io.tile([128, 512], f32, tag="io", name="iops")
                        for ct in range(CT):
                            nc.tensor.matmul(
                                pps[:rows, :ml],
                                lhsT=zc[ct][:, t0 : t0 + rows],
                                rhs=wp[ct][:, m0 : m0 + ml],
                                start=(ct == 0),
                                stop=(ct == CT - 1),
                            )
                        nc.vector.tensor_copy(ob[:rows, m0 : m0 + ml], pps[:rows, :ml])
                    nc.sync.dma_start(out[b, t0 : t0 + rows, :], ob[:rows, :])

                # ---- software-pipelined schedule over the 2 batches ----
                # b0 qkv up front; b1 qkv and b0 proj are interleaved into the
                # exp-throttled pair phases so PE always has filler work and
                # ACT starts exp-ing as early as possible.
                load_x(0)
                for dtt in range(CT):
                    qk_one(0, 0, dtt)
                    qk_one(0, 1, dtt)
                for ti in range(len(TOK)):
                    v_one(0, ti)
                load_x(1)
                for p in range(PAIRS):
                    eps_pair(0, p)
                    qk_one(1, 0, p)
                    qk_one(1, 1, p)
                    if p >= 1:
                        uz_pair(0, p - 1)
                    if p >= 2:
                        v_one(1, p - 2)
                uz_pair(0, PAIRS - 1)
                v_one(1, 4)
                for p in range(PAIRS):
                    eps_pair(1, p)
                    if p >= 1:
                        uz_pair(1, p - 1)
                        proj_one(0, p - 1)
                uz_pair(1, PAIRS - 1)
                for it in range(len(TOK)):
                    proj_one(1, it)

            if loop_n > 1:
                with tc.For_i(0, loop_n, 1):
                    body()
            else:
                body()

    nc.compile()
    return nc


def _prep_in_maps(mode, x, Wq, Wk, Wv, Wp, bp, lamb):
    import ml_dtypes

    D = _dtypes(mode)
    bf16 = ml_dtypes.bfloat16

    def npdt(d):
        import concourse.mybir as mybir

        return bf16 if d == mybir.dt.bfloat16 else np.float32

    npW, npX, npPROJ = npdt(D["W"]), npdt(D["X"]), npdt(D["PROJ"])
    wqT = np.ascontiguousarray(Wq.T).astype(npW)
    wkT = np.ascontiguousarray(Wk.T).astype(npW)
    # fold (1-2l) per head into Wv: V' = (1-2l) V, then
    # W = V' + (3l/(1-2l)) U'/s with U' = E@V'.
    c1f = (1.0 - 2.0 * lamb).astype(np.float32)  # per head, |.| >= 0.3 for N(0,.1) lamb
    vscale = np.repeat(c1f, HD)  # [C], per v-column scale
    wvT = np.ascontiguousarray(Wv.T * vscale[None, :]).astype(npW)
    wpT = np.ascontiguousarray(Wp.T).astype(npPROJ)
    bpr = np.ascontiguousarray(bp.reshape(1, C)).astype(npPROJ)
    c1 = c1f
    c3 = (3.0 * lamb / c1f).astype(np.float32)
    cstv = np.ascontiguousarray(
        np.broadcast_to(np.stack([c1, c3], 0)[None], (128, 2, H))
    ).astype(np.float32)
    onesHv = np.ones((128, H), dtype=npdt(D["ATT"]))
    onesCv = np.ascontiguousarray(
        np.broadcast_to((1.0 / c3)[None, :], (128, H))
    ).astype(npdt(D["ATT"]))
    ones1v = np.ones((1, 128), dtype=npPROJ)
    in_maps = []
    for core in range(NCORES):
        xs = x[core * BPC : (core + 1) * BPC]
        xTv = np.ascontiguousarray(xs.transpose(0, 2, 1)).astype(npX)
        in_maps.append(
            dict(xT=xTv, wqT=wqT, wkT=wkT, wvT=wvT, wpT=wpT, bpr=bpr, cst=cstv,
                 onesH=onesHv, onesC=onesCv, ones1=ones1v)
        )
    return in_maps


def kernel(x, Wq, Wk, Wv, Wp, bp, lamb):
    from concourse.bass_utils import run_bass_kernel_spmd

    x = np.asarray(x, dtype=np.float32)
    Wq = np.asarray(Wq, dtype=np.float32)
    Wk = np.asarray(Wk, dtype=np.float32)
    Wv = np.asarray(Wv, dtype=np.float32)
    Wp = np.asarray(Wp, dtype=np.float32)
    bp = np.asarray(bp, dtype=np.float32)
    lamb = np.asarray(lamb, dtype=np.float32)

    if MODE not in _CACHE:
        _CACHE[MODE] = build(MODE)
    nc = _CACHE[MODE]
    in_maps = _prep_in_maps(MODE, x, Wq, Wk, Wv, Wp, bp, lamb)
    res = run_bass_kernel_spmd(nc, in_maps, list(range(NCORES)))
    out = np.concatenate([res.results[i]["out"] for i in range(NCORES)], axis=0)
    out += bp[None, None, :]
    return out

